# revision 30
# baseline (speedup 1.0000x reference)
"""Trainium2 Bass kernel for nn_Attention_39049842655427.

Multi-head attention (RoPE + hard mask + soft gate mask) over 8
NeuronCores: data-parallel over batch (2) x tensor-parallel over heads
(16 heads -> 4 per core).  Each core computes q/k/v projections for its
4 heads, the head-sharded attention, and a partial output projection
(wo row-sharded); the host sums the 4 partials per batch and adds bo.

Math notes (exact up to float rounding):
  reference:  e = exp(s)*hard ; a1 = e/sum(e) ; a2 = a1*soft
              attn = a2/(sum(a2)+1e-6) ; out = attn @ v
  identity:   attn = f / (F + 1e-6*E),  f = e*hard*soft,
              F = sum(f), E = sum(e*hard)
  kernel:     g = exp(s) * M2,  M2 = hard*(soft+1e-6)
              => sum(g) = F + 1e-6*E exactly; numerator uses g instead
              of f, an O(1e-6) perturbation of attn.
All matmuls run in bf16 with fp32 PSUM accumulation.  Scores are
computed transposed (s[kv,q]) so attn@v needs no on-device transpose;
RoPE pairs are pre-permuted (evens then odds) inside each head's 128
rows of wq/wk so the rotation partner is a partition offset of 64
(applied via a small SBUF->SBUF DMA).  128x512 blocks of M2 that are
exactly zero are skipped entirely (exact, data-adaptive: the keep
pattern is read from the actual mask and baked into the compiled
program; a dense mask falls back to the all-keep program).
"""

import math
import sys
from contextlib import ExitStack

import numpy as np
import ml_dtypes

if "/opt/trn_rl_repo" not in sys.path:
    sys.path.append("/opt/trn_rl_repo")

import concourse.bass as bass  # noqa: E402,F401
import concourse.tile as tile  # noqa: E402
from concourse import bacc, mybir  # noqa: E402
from concourse.bass_utils import run_bass_kernel_spmd  # noqa: E402

B, S, D, H, DK = 2, 2048, 2048, 16, 128
N_CORES = 8
HPC = 4          # heads per core
DSH = HPC * DK   # 512, d-shard per core

BF16 = ml_dtypes.bfloat16

_NC_CACHE = {}


def build_bass(s_len=S, keep=None, col0=None):
    """Build the SPMD single-core program (same NEFF on all 8 cores)."""
    f32 = mybir.dt.float32
    bf16 = mybir.dt.bfloat16
    KC = D // 128          # contraction chunks for projections
    SQ = s_len // 512      # 512-wide q/s chunks
    NKV = s_len // 128     # 128-row kv chunks
    JQ = D // 512          # output-column chunks
    if keep is None:
        keep = tuple(tuple(True for _ in range(NKV)) for _ in range(SQ))
    if col0 is None:
        col0 = tuple(tuple(0 for _ in range(NKV)) for _ in range(SQ))

    nc = bacc.Bacc("TRN2", target_bir_lowering=False, debug=False,
                   num_devices=N_CORES)

    xT = nc.dram_tensor("xT", [SQ, KC // 4, 128, 4, 512], bf16, kind="ExternalInput").ap()
    wqT = nc.dram_tensor("wqT", [KC // 4, 128, 4, DSH], bf16, kind="ExternalInput").ap()
    wkT = nc.dram_tensor("wkT", [KC // 4, 128, 4, DSH], bf16, kind="ExternalInput").ap()
    wvT = nc.dram_tensor("wvT", [KC // 4, 128, 4, DSH], bf16, kind="ExternalInput").ap()
    woT = nc.dram_tensor("woT", [DSH, D], bf16, kind="ExternalInput").ap()
    bqp = nc.dram_tensor("bqp", [128, HPC], f32, kind="ExternalInput").ap()
    bkp = nc.dram_tensor("bkp", [128, HPC], f32, kind="ExternalInput").ap()
    bvb = nc.dram_tensor("bvb", [128, DSH], f32, kind="ExternalInput").ap()
    cosp = nc.dram_tensor("cosp", [128, s_len], bf16, kind="ExternalInput").ap()
    sinp = nc.dram_tensor("sinp", [128, s_len], bf16, kind="ExternalInput").ap()
    m2t = nc.dram_tensor("m2t", [SQ, NKV // 4, 128, 4, 512], bf16, kind="ExternalInput").ap()
    y = nc.dram_tensor("y", [s_len, D], bf16, kind="ExternalOutput").ap()

    Act = mybir.ActivationFunctionType
    inv_sqrt_dk = 1.0 / math.sqrt(DK)

    with tile.TileContext(nc) as tc:
        with (
            tc.tile_pool(name="consts", bufs=1) as consts,
            tc.tile_pool(name="qkv", bufs=1) as qkv,
            tc.tile_pool(name="opool", bufs=2) as opool,
            tc.tile_pool(name="work2", bufs=2) as work2,
            tc.tile_pool(name="ypool", bufs=3) as ypool,
            tc.tile_pool(name="ps_proj", bufs=3, space="PSUM") as ps_proj,
            tc.tile_pool(name="ps_s", bufs=2, space="PSUM") as ps_s,
            tc.tile_pool(name="ps_o", bufs=2, space="PSUM") as ps_o,
            tc.tile_pool(name="ps_d", bufs=1, space="PSUM") as ps_d,
        ):
            # ---- small constants ----
            ones_kv = consts.tile([128, 1], bf16, tag="ones_kv", name="ones_kv")
            nc.vector.memset(ones_kv, 1.0)
            ones_row = consts.tile([1, 128], bf16, tag="ones_row",
                                   name="ones_row")
            nc.vector.memset(ones_row, 1.0)
            # HAM warm-up: ~50 throwaway matmuls while the first DMAs land,
            # so the PE clock gate is open (2.4GHz) when real work arrives
            warm_rhs = consts.tile([128, 512], bf16, tag="warm", name="warm")
            nc.vector.memset(warm_rhs, 0.0)

            # ---- persistent activations (bf16) ----
            qT_sb = [[qkv.tile([128, 512], bf16, tag=f"qT_{h}_{c}", name=f"qT_{h}_{c}")
                      for c in range(SQ)] for h in range(HPC)]
            kT_sb = [[qkv.tile([128, 512], bf16, tag=f"kT_{h}_{c}", name=f"kT_{h}_{c}")
                      for c in range(SQ)] for h in range(HPC)]
            v_sb = [qkv.tile([128, DSH], bf16, tag=f"v_{i}", name=f"v_{i}")
                    for i in range(NKV)]
            wo_sb = [consts.tile([128, D], bf16, tag=f"wo_{h}", name=f"wo_{h}")
                     for h in range(HPC)]
            # m2 mask tiles for q4=0: outer pool so the loads start early
            m2c0 = [qkv.tile([128, 4, 512], bf16, tag=f"m2c0_{i}", name=f"m2c0_{i}")
                    if any(keep[0][4 * i + j] for j in range(4)) else None
                    for i in range(NKV // 4)]
            oT_sb = {}

            # ---------------- emission helpers -----------------
            def emit_attention(q4, m2col, work):
                kept = [kv for kv in range(NKV) if keep[q4][kv]]
                assert kept, "fully masked query column not supported"
                quads = [kept[i:i + 4] for i in range(0, len(kept), 4)]
                oT_sb[q4] = [opool.tile([128, 512], bf16, tag=f"oT_{h}",
                                        name=f"oT_{h}") for h in range(HPC)]
                # first kept block of each q4 must stay full-width so the
                # start=True AV / dsum writes cover the whole psum range
                c0s = {kv: (0 if i == 0 else col0[q4][kv])
                       for i, kv in enumerate(kept)}
                for h in range(HPC):
                    ps_oT = ps_o.tile([128, 512], f32, tag="ps_o", name="ps_o")
                    # row 0 accumulates the D sums; later the whole bank is
                    # overwritten with the broadcast reciprocal (rb matmul)
                    ps_D = ps_d.tile([128, 512], f32, tag="ps_d", name="ps_d")
                    gq = []
                    qi = 0
                    for idx, kv in enumerate(kept):
                        c0 = c0s[kv]
                        cs = slice(c0, 512)
                        ps_sc = ps_s.tile([128, 512], f32, tag="ps_s", name="ps_s")
                        nc.tensor.matmul(
                            ps_sc[:, cs],
                            kT_sb[h][kv // 4][:, (kv % 4) * 128:(kv % 4 + 1) * 128],
                            qT_sb[h][q4][:, cs], start=True, stop=True)
                        e = work.tile([128, 512], bf16, tag="e", name="e")
                        nc.scalar.activation(e[:, cs], ps_sc[:, cs], Act.Exp,
                                             scale=inv_sqrt_dk)
                        g = work.tile([128, 512], bf16, tag=f"g{idx % 4}",
                                      name=f"g{idx % 4}")
                        # stays on DVE: a slow (1.4us) gpsimd mul here gates
                        # the serial AV accumulation chain into ps_oT
                        nc.vector.tensor_mul(g[:, cs], e[:, cs],
                                             m2col[kv // 4][:, kv % 4, cs])
                        nc.tensor.matmul(
                            ps_oT[:, cs], v_sb[kv][:, h * 128:(h + 1) * 128],
                            g[:, cs],
                            start=(idx == 0), stop=(idx == len(kept) - 1))
                        gq.append((g, c0))
                        if len(gq) == len(quads[qi]):
                            # in-place accumulate on DVE into the group's
                            # first (widest) tile; the AV matmul already
                            # consumed these g tiles. 1 D-MM per group,
                            # sliced to the group's valid width (columns left
                            # of it hold stale data, and their true
                            # contribution is exactly zero).
                            acc, ca = gq[0]
                            for gj, cj in gq[1:]:
                                nc.vector.tensor_add(acc[:, cj:512],
                                                     acc[:, cj:512],
                                                     gj[:, cj:512])
                            nc.tensor.matmul(
                                ps_D[0:1, ca:512], ones_kv[:], acc[:, ca:512],
                                start=(qi == 0), stop=(qi == len(quads) - 1))
                            gq = []
                            qi += 1
                    # normalize: oT = ps_oT * (1/D), broadcast along
                    # partitions on gpsimd (its only non-DMA op -> the
                    # broadcast ucode library stays loaded)
                    r_row = work.tile([1, 512], f32, tag="r_row", name="r_row")
                    nc.vector.reciprocal_approx_fast(r_row[:], ps_D[0:1, :])
                    rb = work.tile([128, 512], f32, tag="rb", name="rb")
                    nc.gpsimd.partition_broadcast(rb[:], r_row[:])
                    nc.vector.tensor_mul(oT_sb[q4][h][:], ps_oT[:], rb[:])

            def emit_y(q4):
                # gpsimd cannot read PSUM, so evacuations alternate between
                # ACT (exp-bound) and DVE (mul/add-bound)
                evac = {0: nc.vector.tensor_copy, 1: nc.scalar.copy,
                        2: nc.vector.tensor_copy, 3: nc.scalar.copy}
                dmaq = [nc.sync, nc.scalar, nc.sync, nc.scalar]
                for sl in range(4):
                    srow = slice((q4 * 4 + sl) * 128, (q4 * 4 + sl + 1) * 128)
                    lrow = slice(sl * 128, (sl + 1) * 128)
                    ysb = ypool.tile([128, D], bf16, tag="ysb", name="ysb")
                    for j4 in range(JQ):
                        jcol = slice(j4 * 512, (j4 + 1) * 512)
                        ps_y = ps_proj.tile([128, 512], f32, tag="ps_proj",
                                            name="ps_proj")
                        for h in range(HPC):
                            nc.tensor.matmul(
                                ps_y[:], oT_sb[q4][h][:, lrow], wo_sb[h][:, jcol],
                                start=(h == 0), stop=(h == HPC - 1))
                        evac[j4](ysb[:, jcol], ps_y[:])
                        if q4 == SQ - 1:
                            # tail: dma each 512-col strip as soon as it is
                            # evacuated, spread across all four queues
                            dmaq[(sl + j4) % 4].dma_start(y[srow, jcol],
                                                          ysb[:, jcol])
                    if q4 != SQ - 1:
                        dmaq[sl % 2].dma_start(y[srow, :], ysb[:])

            # warm-up matmuls bridge the ~8us DMA bring-up dead time plus the
            # supply-paced first projection window so the HAM clock gate
            # stays open (2.4GHz) once real work arrives
            ps_warm = ps_s.tile([128, 512], f32, tag="ps_s", name="ps_s")
            for i in range(28):
                nc.tensor.matmul(ps_warm[:], warm_rhs[:, 0:128],
                                 warm_rhs[:], start=(i == 0), stop=(i == 27))

            # ============ phase 1: q/k/v projections (scoped pools) ========
            phase1 = ExitStack()
            wpool = phase1.enter_context(tc.tile_pool(name="wpool", bufs=1))
            xpool = phase1.enter_context(tc.tile_pool(name="xpool", bufs=2))
            work1 = phase1.enter_context(tc.tile_pool(name="work1", bufs=2))
            KP = KC // 4
            wq_sb = [wpool.tile([128, 4, DSH], bf16, tag=f"wq_{i}", name=f"wq_{i}")
                     for i in range(KP)]
            wk_sb = [wpool.tile([128, 4, DSH], bf16, tag=f"wk_{i}", name=f"wk_{i}")
                     for i in range(KP)]
            wv_sb = [wpool.tile([128, 4, DSH], bf16, tag=f"wv_{i}", name=f"wv_{i}")
                     for i in range(KP)]
            xcol0 = [xpool.tile([128, 4, 512], bf16, tag=f"x_{i}", name=f"x_{i}")
                     for i in range(KP)]
            # startup loads, ordered by first-need time so nothing stalls:
            #   scalar: wk (t~0) then wq (t~14us)
            #   sync:   x column 0 (t~0)
            #   gpsimd: bk/cos/sin (K-evac ~14us), bq, bvb, wv (~28us),
            #           m2 col 0 (in-phase-1 attention ~45us)
            #   vector: wo (phase 2 only)
            bk_sb = consts.tile([128, HPC], f32, tag="bk", name="bk")
            nc.gpsimd.dma_start(bk_sb[:], bkp[:])
            cos_sb = consts.tile([128, s_len], bf16, tag="cos", name="cos")
            nc.gpsimd.dma_start(cos_sb[:], cosp[:])
            sin_sb = consts.tile([128, s_len], bf16, tag="sin", name="sin")
            nc.gpsimd.dma_start(sin_sb[:], sinp[:])
            bq_sb = consts.tile([128, HPC], f32, tag="bq", name="bq")
            nc.gpsimd.dma_start(bq_sb[:], bqp[:])
            bvb_sb = consts.tile([128, DSH], f32, tag="bvb", name="bvb")
            nc.gpsimd.dma_start(bvb_sb[:], bvb[:])
            for i in range(KP):
                qa, qb = (nc.sync, nc.scalar) if i % 2 == 0 else (nc.scalar, nc.sync)
                qa.dma_start(xcol0[i][:], xT[0, i])
                qb.dma_start(wk_sb[i][:], wkT[i])
            for i in range(KP):
                (nc.sync if i % 2 == 0 else nc.scalar).dma_start(
                    wq_sb[i][:], wqT[i])
            def emit_bulk_loads():
                # emitted after the s4=0 K projection so the scheduler gives
                # the critical early x/wk/wq transfers the whole DMA engine
                # pool; wv/m2c0/wo are not needed before ~40us
                for i in range(KP):
                    nc.gpsimd.dma_start(wv_sb[i][:], wvT[i])
                for i in range(NKV // 4):
                    if any(keep[0][4 * i + j] for j in range(4)):
                        nc.gpsimd.dma_start(m2c0[i][:], m2t[0, i])
                for h in range(HPC):
                    nc.gpsimd.dma_start(wo_sb[h][:],
                                        woT[h * 128:(h + 1) * 128, :])

            for s4 in range(SQ):
                scol = slice(s4 * 512, (s4 + 1) * 512)
                if s4 == 0:
                    xcol = xcol0
                else:
                    xcol = [xpool.tile([128, 4, 512], bf16, tag=f"x_{i}",
                                       name=f"x_{i}") for i in range(KP)]
                    for i in range(KP):
                        nc.sync.dma_start(xcol[i][:], xT[s4, i])

                # K then Q: out[dk, s] with RoPE (K first: scores read kT)
                for (w_sb, b_sb, dest) in ((wk_sb, bk_sb, kT_sb),
                                           (wq_sb, bq_sb, qT_sb)):
                    for mm in range(HPC):
                        ps = ps_proj.tile([128, 512], f32, tag="ps_proj",
                                          name="ps_proj")
                        for k in range(KC):
                            nc.tensor.matmul(
                                ps[:],
                                w_sb[k // 4][:, k % 4, mm * 128:(mm + 1) * 128],
                                xcol[k // 4][:, k % 4, :],
                                start=(k == 0), stop=(k == KC - 1))
                        q1 = work1.tile([128, 512], bf16, tag="q1", name="q1")
                        nc.scalar.activation(q1[:], ps[:], Act.Identity,
                                             bias=b_sb[:, mm:mm + 1])
                        # pair-swap halves via SBUF->SBUF DMA (partition
                        # shifts are not expressible on DVE/ACT lanes)
                        qsw = work1.tile([128, 512], bf16, tag="qsw", name="qsw")
                        nc.sync.dma_start(qsw[0:64], q1[64:128])
                        nc.sync.dma_start(qsw[64:128], q1[0:64])
                        tsw = work1.tile([128, 512], bf16, tag="tsw", name="tsw")
                        nc.vector.tensor_mul(tsw[:], qsw[:], sin_sb[:, scol])
                        tcs = work1.tile([128, 512], bf16, tag="tcs", name="tcs")
                        nc.vector.tensor_mul(tcs[:], q1[:], cos_sb[:, scol])
                        nc.vector.tensor_add(dest[mm][s4][:], tcs[:], tsw[:])
                    if s4 == 0 and dest is kT_sb:
                        emit_bulk_loads()

                # V: out[s, dk-shard], natural layout
                for sl in range(4):
                    s16 = s4 * 4 + sl
                    ps = ps_proj.tile([128, 512], f32, tag="ps_proj",
                                      name="ps_proj")
                    for k in range(KC):
                        nc.tensor.matmul(
                            ps[:],
                            xcol[k // 4][:, k % 4, sl * 128:(sl + 1) * 128],
                            wv_sb[k // 4][:, k % 4, :],
                            start=(k == 0), stop=(k == KC - 1))
                    nc.vector.tensor_add(v_sb[s16][:], ps[:], bvb_sb[:])

                if s4 == 0 and all(kv < 4 for kv in range(NKV) if keep[0][kv]):
                    # attention for the first q-column only touches s4=0 data
                    # when its keep-pattern is lower-triangular; emitting it
                    # here fills the projection phase's DMA-bound bubbles.
                    emit_attention(0, m2c0, work2)

            phase1.close()

            # ============ phase 2: attention + output projection ============
            phase2 = ExitStack()
            m2pool = phase2.enter_context(tc.tile_pool(name="m2pool", bufs=2))
            work = phase2.enter_context(tc.tile_pool(name="workp2", bufs=4))
            if 0 not in oT_sb:
                emit_attention(0, m2c0, work)
            m2cols = {}
            for q4 in range(1, SQ):
                m2cols[q4] = [m2pool.tile([128, 4, 512], bf16, tag=f"m2_{i}",
                                          name=f"m2_{i}")
                              if any(keep[q4][4 * i + j] for j in range(4))
                              else None for i in range(NKV // 4)]
                for i in range(NKV // 4):
                    if m2cols[q4][i] is not None:
                        # sync queue: keeps phase-2 gpsimd free of DMA
                        # descriptors so its tensor-op ucode stays loaded
                        nc.sync.dma_start(m2cols[q4][i][:], m2t[q4, i])

            for q4 in range(1, SQ):
                emit_attention(q4, m2cols[q4], work)
                emit_y(q4 - 1)
            emit_y(SQ - 1)
            phase2.close()

    nc.compile()
    return nc


def _rope_perm():
    """Within each head's 128 rows: evens first, then odds."""
    base = np.concatenate([np.arange(0, 128, 2), np.arange(1, 128, 2)])
    return np.concatenate([h * 128 + base for h in range(HPC)])


def _blk(a):
    """[R, C] -> [C//512, R//512, 128, 4, 512] packed contiguous blocks.

    Block [c4, i, :, j, :] = a[(4*i+j)*128:(4*i+j+1)*128, c4*512:(c4+1)*512].
    """
    r, c = a.shape
    return np.ascontiguousarray(
        a.reshape(r // 512, 4, 128, c // 512, 512).transpose(3, 0, 2, 1, 4))


def _wpack(a):
    """[R, C] -> [R//512, 128, 4, C]: pack 4 row-chunks per tile."""
    r, c = a.shape
    return np.ascontiguousarray(
        a.reshape(r // 512, 4, 128, c).transpose(0, 2, 1, 3))


def prepare_inputs(x, freqs, hard_mask, soft_mask, wq, bq, wk, bk, wv, bv, wo,
                   s_len=S):
    """Host-side shard + layout prep.  Returns one in_map per core."""
    perm = _rope_perm()
    cos = np.cos(np.asarray(freqs, np.float32))   # [S, 64]
    sin = np.sin(np.asarray(freqs, np.float32))
    cosp = np.ascontiguousarray(
        np.concatenate([cos.T, cos.T], axis=0)).astype(BF16)     # [128, S]
    sinp = np.ascontiguousarray(
        np.concatenate([-sin.T, sin.T], axis=0)).astype(BF16)
    hard = np.asarray(hard_mask, np.float32).reshape(s_len, s_len)
    soft = np.asarray(soft_mask, np.float32).reshape(s_len, s_len)
    m2t = _blk((hard * (soft + 1e-6)).T.astype(BF16))

    xT = [_blk(np.asarray(x[b], np.float32).T.astype(BF16)) for b in range(B)]

    per_group = []
    for hg in range(4):
        rows = slice(DSH * hg, DSH * (hg + 1))
        wq_sh = np.asarray(wq, np.float32)[rows][perm]
        wk_sh = np.asarray(wk, np.float32)[rows][perm]
        wv_sh = np.asarray(wv, np.float32)[rows]
        per_group.append({
            "wqT": _wpack(np.ascontiguousarray(wq_sh.T).astype(BF16)),
            "wkT": _wpack(np.ascontiguousarray(wk_sh.T).astype(BF16)),
            "wvT": _wpack(np.ascontiguousarray(wv_sh.T).astype(BF16)),
            "woT": np.ascontiguousarray(
                np.asarray(wo, np.float32)[:, rows].T).astype(BF16),
            "bqp": np.ascontiguousarray(
                np.asarray(bq, np.float32)[rows][perm].reshape(HPC, 128).T),
            "bkp": np.ascontiguousarray(
                np.asarray(bk, np.float32)[rows][perm].reshape(HPC, 128).T),
            "bvb": np.ascontiguousarray(np.broadcast_to(
                np.asarray(bv, np.float32)[rows][None, :], (128, DSH))),
        })

    in_maps = []
    for core in range(N_CORES):
        b, hg = core // 4, core % 4
        m = {"xT": xT[b], "cosp": cosp, "sinp": sinp, "m2t": m2t}
        m.update(per_group[hg])
        in_maps.append(m)
    return in_maps


def kernel(x, freqs, hard_mask, soft_mask, wq, bq, wk, bk, wv, bv, wo, bo,
           _trace=False, _tmpdir=None):
    s_len = x.shape[1]
    in_maps = prepare_inputs(x, freqs, hard_mask, soft_mask, wq, bq, wk, bk,
                             wv, bv, wo, s_len=s_len)
    m2b = in_maps[0]["m2t"]  # [SQ, NKV//4, 128, 4, 512]
    keep = tuple(tuple(bool(np.any(m2b[q4, kv // 4, :, kv % 4] != 0))
                       for kv in range(m2b.shape[1] * 4))
                 for q4 in range(m2b.shape[0]))
    # first query column with any unmasked element, per kept block (the
    # columns left of it are exactly zero in m2 -> skipped everywhere)
    def _c0(q4, kv):
        if not keep[q4][kv]:
            return 0
        nz = np.flatnonzero(np.any(m2b[q4, kv // 4, :, kv % 4] != 0, axis=0))
        return int(nz[0]) if len(nz) else 0
    col0 = tuple(tuple(_c0(q4, kv) for kv in range(m2b.shape[1] * 4))
                 for q4 in range(m2b.shape[0]))
    ckey = (s_len, keep, col0)
    if ckey not in _NC_CACHE:
        _NC_CACHE[ckey] = build_bass(s_len, keep, col0)
    nc = _NC_CACHE[ckey]
    kwargs = {}
    if _trace:
        kwargs = {"trace": True, "tmpdir": _tmpdir}
    res = run_bass_kernel_spmd(nc, in_maps, core_ids=list(range(N_CORES)),
                               **kwargs)
    bo32 = np.asarray(bo, np.float32)
    out = np.empty((B, s_len, D), np.float32)
    for b in range(B):
        acc = res.results[4 * b]["y"].astype(np.float32)
        for hg in range(1, 4):
            acc = acc + res.results[4 * b + hg]["y"].astype(np.float32)
        out[b] = acc + bo32[None, :]
    kernel.last_result = res
    return out



# revision 40
# speedup vs baseline: 1.1749x; 1.1749x over previous
"""Trainium2 Bass kernel for nn_Attention_39049842655427.

Multi-head attention (RoPE + hard mask + soft gate mask) over 8
NeuronCores: data-parallel over batch (2) x tensor-parallel over heads
(16 heads -> 4 per core).  Each core computes q/k/v projections for its
4 heads, the head-sharded attention, and a partial output projection
(wo row-sharded); the host sums the 4 partials per batch and adds bo.

Math notes (exact up to float rounding):
  reference:  e = exp(s)*hard ; a1 = e/sum(e) ; a2 = a1*soft
              attn = a2/(sum(a2)+1e-6) ; out = attn @ v
  identity:   attn = f / (F + 1e-6*E),  f = e*hard*soft,
              F = sum(f), E = sum(e*hard)
  kernel:     g = exp(s) * M2,  M2 = hard*(soft+1e-6)
              => sum(g) = F + 1e-6*E exactly; numerator uses g instead
              of f, an O(1e-6) perturbation of attn.
All matmuls run in bf16 with fp32 PSUM accumulation.  Scores are
computed transposed (s[kv,q]) so attn@v needs no on-device transpose;
RoPE pairs are pre-permuted (evens then odds) inside each head's 128
rows of wq/wk so the rotation partner is a partition offset of 64
(applied via a small SBUF->SBUF DMA).  128x512 blocks of M2 that are
exactly zero are skipped entirely (exact, data-adaptive: the keep
pattern is read from the actual mask and baked into the compiled
program; a dense mask falls back to the all-keep program).
"""

import math
import sys
from contextlib import ExitStack

import numpy as np
import ml_dtypes

if "/opt/trn_rl_repo" not in sys.path:
    sys.path.append("/opt/trn_rl_repo")

import concourse.bass as bass  # noqa: E402,F401
import concourse.tile as tile  # noqa: E402
from concourse import bacc, mybir  # noqa: E402
from concourse.bass_utils import run_bass_kernel_spmd  # noqa: E402

B, S, D, H, DK = 2, 2048, 2048, 16, 128
N_CORES = 8
HPC = 4          # heads per core
DSH = HPC * DK   # 512, d-shard per core

BF16 = ml_dtypes.bfloat16

_NC_CACHE = {}


def build_bass(s_len=S, keep=None, col0=None):
    """Build the SPMD single-core program (same NEFF on all 8 cores)."""
    f32 = mybir.dt.float32
    bf16 = mybir.dt.bfloat16
    KC = D // 128          # contraction chunks for projections
    SQ = s_len // 512      # 512-wide q/s chunks
    NKV = s_len // 128     # 128-row kv chunks
    JQ = D // 512          # output-column chunks
    if keep is None:
        keep = tuple(tuple(True for _ in range(NKV)) for _ in range(SQ))
    if col0 is None:
        col0 = tuple(tuple(0 for _ in range(NKV)) for _ in range(SQ))

    nc = bacc.Bacc("TRN2", target_bir_lowering=False, debug=False,
                   num_devices=N_CORES)

    xT = nc.dram_tensor("xT", [SQ, KC // 4, 128, 4, 512], bf16, kind="ExternalInput").ap()
    wqT = nc.dram_tensor("wqT", [KC // 4, 128, 4, DSH], bf16, kind="ExternalInput").ap()
    wkT = nc.dram_tensor("wkT", [KC // 4, 128, 4, DSH], bf16, kind="ExternalInput").ap()
    wvT = nc.dram_tensor("wvT", [KC // 4, 128, 4, DSH], bf16, kind="ExternalInput").ap()
    woT = nc.dram_tensor("woT", [DSH, D], bf16, kind="ExternalInput").ap()
    bqp = nc.dram_tensor("bqp", [128, HPC], f32, kind="ExternalInput").ap()
    bkp = nc.dram_tensor("bkp", [128, HPC], f32, kind="ExternalInput").ap()
    bvb = nc.dram_tensor("bvb", [128, DSH], f32, kind="ExternalInput").ap()
    cosp = nc.dram_tensor("cosp", [128, s_len], bf16, kind="ExternalInput").ap()
    sinp = nc.dram_tensor("sinp", [128, s_len], bf16, kind="ExternalInput").ap()
    m2t = nc.dram_tensor("m2t", [SQ, NKV // 4, 128, 4, 512], bf16, kind="ExternalInput").ap()
    y = nc.dram_tensor("y", [s_len, D], bf16, kind="ExternalOutput").ap()

    Act = mybir.ActivationFunctionType
    inv_sqrt_dk = 1.0 / math.sqrt(DK)

    with tile.TileContext(nc) as tc:
        with (
            tc.tile_pool(name="consts", bufs=1) as consts,
            tc.tile_pool(name="qkv", bufs=1) as qkv,
            tc.tile_pool(name="opool", bufs=2) as opool,
            tc.tile_pool(name="work2", bufs=2) as work2,
            tc.tile_pool(name="ypool", bufs=2) as ypool,
            tc.tile_pool(name="ps_proj", bufs=3, space="PSUM") as ps_proj,
            tc.tile_pool(name="ps_s", bufs=2, space="PSUM") as ps_s,
            tc.tile_pool(name="ps_o", bufs=2, space="PSUM") as ps_o,
            tc.tile_pool(name="ps_d", bufs=1, space="PSUM") as ps_d,
        ):
            # ---- small constants ----
            ones_kv = consts.tile([128, 1], bf16, tag="ones_kv", name="ones_kv")
            nc.vector.memset(ones_kv, 1.0)
            ones_row = consts.tile([1, 128], bf16, tag="ones_row",
                                   name="ones_row")
            nc.vector.memset(ones_row, 1.0)
            # HAM warm-up: ~50 throwaway matmuls while the first DMAs land,
            # so the PE clock gate is open (2.4GHz) when real work arrives
            warm_rhs = consts.tile([128, 512], bf16, tag="warm", name="warm")
            nc.vector.memset(warm_rhs, 0.0)

            # ---- persistent activations (bf16) ----
            qT_sb = [[qkv.tile([128, 512], bf16, tag=f"qT_{h}_{c}", name=f"qT_{h}_{c}")
                      for c in range(SQ)] for h in range(HPC)]
            kT_sb = [[qkv.tile([128, 512], bf16, tag=f"kT_{h}_{c}", name=f"kT_{h}_{c}")
                      for c in range(SQ)] for h in range(HPC)]
            v_sb = [qkv.tile([128, DSH], bf16, tag=f"v_{i}", name=f"v_{i}")
                    for i in range(NKV)]
            wo_sb = [consts.tile([128, D], bf16, tag=f"wo_{h}", name=f"wo_{h}")
                     for h in range(HPC)]
            # m2 mask tiles for q4=0/1: attention for those query columns is
            # hoisted into the projection phase (its exp/mul work rides the
            # idle ACT/DVE capacity there)
            hoist0 = all(kv < 4 for kv in range(NKV) if keep[0][kv])
            hoist1 = (SQ > 1 and hoist0
                      and all(kv < 8 for kv in range(NKV) if keep[1][kv]))
            m2c0 = [qkv.tile([128, 4, 512], bf16, tag=f"m2c0_{i}", name=f"m2c0_{i}")
                    if any(keep[0][4 * i + j] for j in range(4)) else None
                    for i in range(NKV // 4)]
            m2c1 = [qkv.tile([128, 4, 512], bf16, tag=f"m2c1_{i}", name=f"m2c1_{i}")
                    if any(keep[1][4 * i + j] for j in range(4)) else None
                    for i in range(NKV // 4)] if hoist1 else None
            oT_sb = {}
            y_done = set()

            # ---------------- emission helpers -----------------
            def emit_attention(q4, m2col, work):
                kept = [kv for kv in range(NKV) if keep[q4][kv]]
                assert kept, "fully masked query column not supported"
                quads = [kept[i:i + 4] for i in range(0, len(kept), 4)]
                oT_sb[q4] = [opool.tile([128, 512], bf16, tag=f"oT_{h}",
                                        name=f"oT_{h}") for h in range(HPC)]
                # first kept block of each q4 must stay full-width so the
                # start=True AV / dsum writes cover the whole psum range
                c0s = {kv: (0 if i == 0 else col0[q4][kv])
                       for i, kv in enumerate(kept)}
                for h in range(HPC):
                    ps_oT = ps_o.tile([128, 512], f32, tag="ps_o", name="ps_o")
                    # row 0 accumulates the D sums; later the whole bank is
                    # overwritten with the broadcast reciprocal (rb matmul)
                    ps_D = ps_d.tile([128, 512], f32, tag="ps_d", name="ps_d")
                    gq = []
                    qi = 0
                    for idx, kv in enumerate(kept):
                        c0 = c0s[kv]
                        cs = slice(c0, 512)
                        ps_sc = ps_s.tile([128, 512], f32, tag="ps_s", name="ps_s")
                        nc.tensor.matmul(
                            ps_sc[:, cs],
                            kT_sb[h][kv // 4][:, (kv % 4) * 128:(kv % 4 + 1) * 128],
                            qT_sb[h][q4][:, cs], start=True, stop=True)
                        e = work.tile([128, 512], bf16, tag="e", name="e")
                        nc.scalar.activation(e[:, cs], ps_sc[:, cs], Act.Exp,
                                             scale=inv_sqrt_dk)
                        g = work.tile([128, 512], bf16, tag=f"g{idx % 4}",
                                      name=f"g{idx % 4}")
                        # stays on DVE: a slow (1.4us) gpsimd mul here gates
                        # the serial AV accumulation chain into ps_oT
                        nc.vector.tensor_mul(g[:, cs], e[:, cs],
                                             m2col[kv // 4][:, kv % 4, cs])
                        nc.tensor.matmul(
                            ps_oT[:, cs], v_sb[kv][:, h * 128:(h + 1) * 128],
                            g[:, cs],
                            start=(idx == 0), stop=(idx == len(kept) - 1))
                        gq.append((g, c0))
                        if len(gq) == len(quads[qi]):
                            # in-place accumulate on DVE into the group's
                            # first (widest) tile; the AV matmul already
                            # consumed these g tiles. 1 D-MM per group,
                            # sliced to the group's valid width (columns left
                            # of it hold stale data, and their true
                            # contribution is exactly zero).
                            acc, ca = gq[0]
                            for gj, cj in gq[1:]:
                                nc.vector.tensor_add(acc[:, cj:512],
                                                     acc[:, cj:512],
                                                     gj[:, cj:512])
                            nc.tensor.matmul(
                                ps_D[0:1, ca:512], ones_kv[:], acc[:, ca:512],
                                start=(qi == 0), stop=(qi == len(quads) - 1))
                            gq = []
                            qi += 1
                    # normalize: oT = ps_oT * (1/D), broadcast along
                    # partitions on gpsimd (its only non-DMA op -> the
                    # broadcast ucode library stays loaded)
                    r_row = work.tile([1, 512], f32, tag="r_row", name="r_row")
                    nc.vector.reciprocal_approx_fast(r_row[:], ps_D[0:1, :])
                    rb = work.tile([128, 512], f32, tag="rb", name="rb")
                    nc.gpsimd.partition_broadcast(rb[:], r_row[:])
                    nc.vector.tensor_mul(oT_sb[q4][h][:], ps_oT[:], rb[:])

            def emit_y(q4):
                # gpsimd cannot read PSUM, so evacuations alternate between
                # ACT (exp-bound) and DVE (mul/add-bound)
                evac = {0: nc.vector.tensor_copy, 1: nc.scalar.copy,
                        2: nc.vector.tensor_copy, 3: nc.scalar.copy}
                dmaq = [nc.sync, nc.scalar, nc.sync, nc.scalar]
                for sl in range(4):
                    srow = slice((q4 * 4 + sl) * 128, (q4 * 4 + sl + 1) * 128)
                    lrow = slice(sl * 128, (sl + 1) * 128)
                    ysb = ypool.tile([128, D], bf16, tag="ysb", name="ysb")
                    for j4 in range(JQ):
                        jcol = slice(j4 * 512, (j4 + 1) * 512)
                        ps_y = ps_proj.tile([128, 512], f32, tag="ps_proj",
                                            name="ps_proj")
                        for h in range(HPC):
                            nc.tensor.matmul(
                                ps_y[:], oT_sb[q4][h][:, lrow], wo_sb[h][:, jcol],
                                start=(h == 0), stop=(h == HPC - 1))
                        evac[j4](ysb[:, jcol], ps_y[:])
                        if q4 == SQ - 1:
                            # tail: dma each 512-col strip as soon as it is
                            # evacuated, spread across all four queues
                            dmaq[(sl + j4) % 4].dma_start(y[srow, jcol],
                                                          ysb[:, jcol])
                    if q4 != SQ - 1:
                        dmaq[sl % 2].dma_start(y[srow, :], ysb[:])

            # warm-up matmuls bridge the ~8us DMA bring-up dead time plus the
            # supply-paced first projection window so the HAM clock gate
            # stays open (2.4GHz) once real work arrives
            ps_warm = ps_s.tile([128, 512], f32, tag="ps_s", name="ps_s")
            for i in range(28):
                nc.tensor.matmul(ps_warm[:], warm_rhs[:, 0:128],
                                 warm_rhs[:], start=(i == 0), stop=(i == 27))

            # ============ phase 1: q/k/v projections (scoped pools) ========
            phase1 = ExitStack()
            wpool = phase1.enter_context(tc.tile_pool(name="wpool", bufs=1))
            xpool = phase1.enter_context(tc.tile_pool(name="xpool", bufs=2))
            work1 = phase1.enter_context(tc.tile_pool(name="work1", bufs=2))
            KP = KC // 4
            wq_sb = [wpool.tile([128, 4, DSH], bf16, tag=f"wq_{i}", name=f"wq_{i}")
                     for i in range(KP)]
            wk_sb = [wpool.tile([128, 4, DSH], bf16, tag=f"wk_{i}", name=f"wk_{i}")
                     for i in range(KP)]
            wv_sb = [wpool.tile([128, 4, DSH], bf16, tag=f"wv_{i}", name=f"wv_{i}")
                     for i in range(KP)]
            xcol0 = [xpool.tile([128, 4, 512], bf16, tag=f"x_{i}", name=f"x_{i}")
                     for i in range(KP)]
            # startup loads, ordered by first-need time so nothing stalls:
            #   scalar: wk (t~0) then wq (t~14us)
            #   sync:   x column 0 (t~0)
            #   gpsimd: bk/cos/sin (K-evac ~14us), bq, bvb, wv (~28us),
            #           m2 col 0 (in-phase-1 attention ~45us)
            #   vector: wo (phase 2 only)
            bk_sb = consts.tile([128, HPC], f32, tag="bk", name="bk")
            nc.gpsimd.dma_start(bk_sb[:], bkp[:])
            cos_sb = consts.tile([128, s_len], bf16, tag="cos", name="cos")
            nc.gpsimd.dma_start(cos_sb[:], cosp[:])
            sin_sb = consts.tile([128, s_len], bf16, tag="sin", name="sin")
            nc.gpsimd.dma_start(sin_sb[:], sinp[:])
            bq_sb = consts.tile([128, HPC], f32, tag="bq", name="bq")
            nc.gpsimd.dma_start(bq_sb[:], bqp[:])
            bvb_sb = consts.tile([128, DSH], f32, tag="bvb", name="bvb")
            nc.gpsimd.dma_start(bvb_sb[:], bvb[:])
            for i in range(KP):
                qa, qb = (nc.sync, nc.scalar) if i % 2 == 0 else (nc.scalar, nc.sync)
                qa.dma_start(xcol0[i][:], xT[0, i])
                qb.dma_start(wk_sb[i][:], wkT[i])
            for i in range(KP):
                (nc.sync if i % 2 == 0 else nc.scalar).dma_start(
                    wq_sb[i][:], wqT[i])
            def emit_bulk_loads():
                # emitted after the s4=0 K projection so the scheduler gives
                # the critical early x/wk/wq transfers the whole DMA engine
                # pool; wv/m2c0/wo are not needed before ~40us
                for i in range(KP):
                    nc.gpsimd.dma_start(wv_sb[i][:], wvT[i])
                for i in range(NKV // 4):
                    if any(keep[0][4 * i + j] for j in range(4)):
                        nc.gpsimd.dma_start(m2c0[i][:], m2t[0, i])
                for i in range(NKV // 4):
                    if hoist1 and any(keep[1][4 * i + j] for j in range(4)):
                        nc.gpsimd.dma_start(m2c1[i][:], m2t[1, i])
                for h in range(HPC):
                    nc.gpsimd.dma_start(wo_sb[h][:],
                                        woT[h * 128:(h + 1) * 128, :])

            for s4 in range(SQ):
                scol = slice(s4 * 512, (s4 + 1) * 512)
                if s4 == 0:
                    xcol = xcol0
                else:
                    xcol = [xpool.tile([128, 4, 512], bf16, tag=f"x_{i}",
                                       name=f"x_{i}") for i in range(KP)]
                    for i in range(KP):
                        nc.sync.dma_start(xcol[i][:], xT[s4, i])

                # K then Q: out[dk, s] with RoPE (K first: scores read kT)
                for (w_sb, b_sb, dest) in ((wk_sb, bk_sb, kT_sb),
                                           (wq_sb, bq_sb, qT_sb)):
                    for mm in range(HPC):
                        ps = ps_proj.tile([128, 512], f32, tag="ps_proj",
                                          name="ps_proj")
                        for k in range(KC):
                            nc.tensor.matmul(
                                ps[:],
                                w_sb[k // 4][:, k % 4, mm * 128:(mm + 1) * 128],
                                xcol[k // 4][:, k % 4, :],
                                start=(k == 0), stop=(k == KC - 1))
                        q1 = work1.tile([128, 512], bf16, tag="q1", name="q1")
                        nc.scalar.activation(q1[:], ps[:], Act.Identity,
                                             bias=b_sb[:, mm:mm + 1])
                        # pair-swap halves via SBUF->SBUF DMA (partition
                        # shifts are not expressible on DVE/ACT lanes)
                        qsw = work1.tile([128, 512], bf16, tag="qsw", name="qsw")
                        nc.sync.dma_start(qsw[0:64], q1[64:128])
                        nc.sync.dma_start(qsw[64:128], q1[0:64])
                        # in-place muls keep work1 at two tags (SBUF budget)
                        nc.vector.tensor_mul(qsw[:], qsw[:], sin_sb[:, scol])
                        nc.vector.tensor_mul(q1[:], q1[:], cos_sb[:, scol])
                        nc.vector.tensor_add(dest[mm][s4][:], q1[:], qsw[:])
                    if s4 == 0 and dest is kT_sb:
                        emit_bulk_loads()

                # V: out[s, dk-shard], natural layout
                for sl in range(4):
                    s16 = s4 * 4 + sl
                    ps = ps_proj.tile([128, 512], f32, tag="ps_proj",
                                      name="ps_proj")
                    for k in range(KC):
                        nc.tensor.matmul(
                            ps[:],
                            xcol[k // 4][:, k % 4, sl * 128:(sl + 1) * 128],
                            wv_sb[k // 4][:, k % 4, :],
                            start=(k == 0), stop=(k == KC - 1))
                    nc.vector.tensor_add(v_sb[s16][:], ps[:], bvb_sb[:])

                if s4 == 0 and hoist0:
                    # attention for the first q-column only touches s4=0 data
                    # when its keep-pattern is lower-triangular; emitting it
                    # here fills the projection phase's DMA-bound bubbles.
                    emit_attention(0, m2c0, work2)
                if s4 == 1 and hoist1 and 0 in oT_sb:
                    # q-column 1 likewise only needs s4<=1 projections; its
                    # exp/mul/add work rides phase-1's idle ACT/DVE, and the
                    # first output-projection column follows right behind
                    emit_attention(1, m2c1, work2)
                    emit_y(0)
                    y_done.add(0)

            phase1.close()

            # ============ phase 2: attention + output projection ============
            phase2 = ExitStack()
            m2pool = phase2.enter_context(tc.tile_pool(name="m2pool", bufs=2))
            work = phase2.enter_context(tc.tile_pool(name="workp2", bufs=4))
            if 0 not in oT_sb:
                emit_attention(0, m2c0, work)
            m2cols = {}
            for q4 in range(1, SQ):
                if q4 in oT_sb:
                    continue
                m2cols[q4] = [m2pool.tile([128, 4, 512], bf16, tag=f"m2_{i}",
                                          name=f"m2_{i}")
                              if any(keep[q4][4 * i + j] for j in range(4))
                              else None for i in range(NKV // 4)]
                for i in range(NKV // 4):
                    if m2cols[q4][i] is not None:
                        # sync queue: keeps phase-2 gpsimd free of DMA
                        # descriptors so its tensor-op ucode stays loaded
                        nc.sync.dma_start(m2cols[q4][i][:], m2t[q4, i])

            for q4 in range(1, SQ):
                if q4 not in oT_sb:
                    emit_attention(q4, m2cols[q4], work)
                if q4 - 1 not in y_done:
                    emit_y(q4 - 1)
                    y_done.add(q4 - 1)
            for q4 in range(SQ):
                if q4 not in y_done:
                    emit_y(q4)
                    y_done.add(q4)
            phase2.close()

    nc.compile()
    return nc


def _rope_perm():
    """Within each head's 128 rows: evens first, then odds."""
    base = np.concatenate([np.arange(0, 128, 2), np.arange(1, 128, 2)])
    return np.concatenate([h * 128 + base for h in range(HPC)])


def _blk(a):
    """[R, C] -> [C//512, R//512, 128, 4, 512] packed contiguous blocks.

    Block [c4, i, :, j, :] = a[(4*i+j)*128:(4*i+j+1)*128, c4*512:(c4+1)*512].
    """
    r, c = a.shape
    return np.ascontiguousarray(
        a.reshape(r // 512, 4, 128, c // 512, 512).transpose(3, 0, 2, 1, 4))


def _wpack(a):
    """[R, C] -> [R//512, 128, 4, C]: pack 4 row-chunks per tile."""
    r, c = a.shape
    return np.ascontiguousarray(
        a.reshape(r // 512, 4, 128, c).transpose(0, 2, 1, 3))


def prepare_inputs(x, freqs, hard_mask, soft_mask, wq, bq, wk, bk, wv, bv, wo,
                   s_len=S):
    """Host-side shard + layout prep.  Returns one in_map per core."""
    perm = _rope_perm()
    cos = np.cos(np.asarray(freqs, np.float32))   # [S, 64]
    sin = np.sin(np.asarray(freqs, np.float32))
    cosp = np.ascontiguousarray(
        np.concatenate([cos.T, cos.T], axis=0)).astype(BF16)     # [128, S]
    sinp = np.ascontiguousarray(
        np.concatenate([-sin.T, sin.T], axis=0)).astype(BF16)
    hard = np.asarray(hard_mask, np.float32).reshape(s_len, s_len)
    soft = np.asarray(soft_mask, np.float32).reshape(s_len, s_len)
    m2t = _blk((hard * (soft + 1e-6)).T.astype(BF16))

    xT = [_blk(np.asarray(x[b], np.float32).T.astype(BF16)) for b in range(B)]

    per_group = []
    for hg in range(4):
        rows = slice(DSH * hg, DSH * (hg + 1))
        wq_sh = np.asarray(wq, np.float32)[rows][perm]
        wk_sh = np.asarray(wk, np.float32)[rows][perm]
        wv_sh = np.asarray(wv, np.float32)[rows]
        per_group.append({
            "wqT": _wpack(np.ascontiguousarray(wq_sh.T).astype(BF16)),
            "wkT": _wpack(np.ascontiguousarray(wk_sh.T).astype(BF16)),
            "wvT": _wpack(np.ascontiguousarray(wv_sh.T).astype(BF16)),
            "woT": np.ascontiguousarray(
                np.asarray(wo, np.float32)[:, rows].T).astype(BF16),
            "bqp": np.ascontiguousarray(
                np.asarray(bq, np.float32)[rows][perm].reshape(HPC, 128).T),
            "bkp": np.ascontiguousarray(
                np.asarray(bk, np.float32)[rows][perm].reshape(HPC, 128).T),
            "bvb": np.ascontiguousarray(np.broadcast_to(
                np.asarray(bv, np.float32)[rows][None, :], (128, DSH))),
        })

    in_maps = []
    for core in range(N_CORES):
        b, hg = core // 4, core % 4
        m = {"xT": xT[b], "cosp": cosp, "sinp": sinp, "m2t": m2t}
        m.update(per_group[hg])
        in_maps.append(m)
    return in_maps


def kernel(x, freqs, hard_mask, soft_mask, wq, bq, wk, bk, wv, bv, wo, bo,
           _trace=False, _tmpdir=None):
    s_len = x.shape[1]
    in_maps = prepare_inputs(x, freqs, hard_mask, soft_mask, wq, bq, wk, bk,
                             wv, bv, wo, s_len=s_len)
    m2b = in_maps[0]["m2t"]  # [SQ, NKV//4, 128, 4, 512]
    keep = tuple(tuple(bool(np.any(m2b[q4, kv // 4, :, kv % 4] != 0))
                       for kv in range(m2b.shape[1] * 4))
                 for q4 in range(m2b.shape[0]))
    # first query column with any unmasked element, per kept block (the
    # columns left of it are exactly zero in m2 -> skipped everywhere)
    def _c0(q4, kv):
        if not keep[q4][kv]:
            return 0
        nz = np.flatnonzero(np.any(m2b[q4, kv // 4, :, kv % 4] != 0, axis=0))
        return int(nz[0]) if len(nz) else 0
    col0 = tuple(tuple(_c0(q4, kv) for kv in range(m2b.shape[1] * 4))
                 for q4 in range(m2b.shape[0]))
    ckey = (s_len, keep, col0)
    if ckey not in _NC_CACHE:
        _NC_CACHE[ckey] = build_bass(s_len, keep, col0)
    nc = _NC_CACHE[ckey]
    kwargs = {}
    if _trace:
        kwargs = {"trace": True, "tmpdir": _tmpdir}
    res = run_bass_kernel_spmd(nc, in_maps, core_ids=list(range(N_CORES)),
                               **kwargs)
    bo32 = np.asarray(bo, np.float32)
    out = np.empty((B, s_len, D), np.float32)
    for b in range(B):
        acc = res.results[4 * b]["y"].astype(np.float32)
        for hg in range(1, 4):
            acc = acc + res.results[4 * b + hg]["y"].astype(np.float32)
        out[b] = acc + bo32[None, :]
    kernel.last_result = res
    return out

